# revision 15
# baseline (speedup 1.0000x reference)
"""GroupNorm + single-head-per-core attention + output projection for
nn_Attention_55697135894780 on 8 TRN2 NeuronCores.

Sharding: one (batch, head) pair per core (B=2 x NH=4 = 8 cores), no
cross-device communication. Each core computes, for its (b, h):

  xc     = raw x[b] in bf16 with a ones row (row 64), straight from DMA.
           GroupNorm's mean subtraction never touches the big tensor:
           it is folded into per-output-channel correction columns
           (corr = W_scaled @ mean - bias_row, one tiny matmul per
           projection) applied during the PSUM->SBUF copies (ScalarE
           Identity-bias / VectorE tensor_scalar-subtract), and for the
           G path deferred to the host via num/den linearity.
  s_c    = gn_weight_c / sqrt(group_var_c + eps); group sums via one PE
           matmul against a group-membership matrix (no shuffles).
  q4/k4  = replicated head projections                    [128, 3072] bf16
  S^T    = K^T Q computed j-on-partitions (no transposes anywhere)
  E      = exp(S^T) in bf16, split between ScalarE (table exp) and
           VectorE (one-instruction Schraudolph bit-trick) streaming
           concurrently from different PSUM banks
  out    = [w_out_h @ w_v_h @ norm ; 1]^T-weighted sum of E
           rows 0:64 = numerator, row 64 = softmax denominator;
           col N carries the host-side G correction vector.

Host combines: x + b_out + sum_h(num/den - corr_g), reshaped.
"""

import sys
from contextlib import ExitStack

import numpy as np
import ml_dtypes

sys.path.insert(0, "/opt/trn_rl_repo")

import concourse.bacc as bacc  # noqa: E402
import concourse.bass as bass  # noqa: E402
import concourse.tile as tile  # noqa: E402
from concourse import mybir  # noqa: E402
from concourse.bass_utils import run_bass_kernel_spmd  # noqa: E402

B, C, D_, H_, W_ = 2, 64, 12, 16, 16
N = D_ * H_ * W_  # 3072
NH, DH, NG = 4, 16, 4  # heads, head_dim, groups
EPS = 1e-5
F32 = mybir.dt.float32
BF16 = mybir.dt.bfloat16
I32 = mybir.dt.int32
I16 = mybir.dt.int16
ALU = mybir.AluOpType
ACTF = mybir.ActivationFunctionType

NCHUNK = 512
NCH = N // NCHUNK  # 6 i-chunks
JBLK = 128
NJB = N // JBLK  # 24 j-blocks
PACK = 3  # j-blocks per PSUM pack (3 banks; x2 buffers + 2 PV banks = 8)
NPACKS = NJB // PACK  # 8 packs per chunk, 48 total
NWARM = 26

# Global pack indices whose exp runs on VectorE (Schraudolph). Pattern
# A D A D A per 5 keeps adjacent packs mostly on different engines so the
# 2-slot PSUM rotation lets both engines stream concurrently.
DVE_PACKS = frozenset(g for g in range(NCH * NPACKS) if g % 5 in (1, 3))

# Schraudolph constants: bits of bf16(exp(S)) ~= int16(S*128/ln2 + 127*128 - CSH)
ASH = 128.0 / float(np.log(2.0))
CSH = 5.5
BSH = 127.0 * 128.0 - CSH

# const-block column layout (fp32 [65, 386]):
#   0:128 wq4 (row64 = hq), 128:256 wk4 (row64 = hk),
#   256:320 mvoT (row64 = hg), 320 gnw (row64 = 1), 321 pad,
#   322:386 gmat (group-membership 0/1, row64 = 0)
CB_W = 386
OUT_W = N + 1  # col N = corr_g vector (rows 0:64)


def build_program():
    nc = bacc.Bacc("TRN2", target_bir_lowering=False)

    xb_d = nc.dram_tensor("xb", [C + 1, N], BF16, kind="ExternalInput")
    cblk_d = nc.dram_tensor("cblk", [C + 1, CB_W], F32, kind="ExternalInput")
    out_d = nc.dram_tensor("out", [C + 1, OUT_W], F32, kind="ExternalOutput")

    with tile.TileContext(nc) as tc, ExitStack() as ctx:
        consts = ctx.enter_context(tc.tile_pool(name="consts", bufs=1))
        work = ctx.enter_context(tc.tile_pool(name="work", bufs=1))
        small = ctx.enter_context(tc.tile_pool(name="small", bufs=2))
        epool = ctx.enter_context(tc.tile_pool(name="epool", bufs=5))
        opool = ctx.enter_context(tc.tile_pool(name="opool", bufs=2))
        psum = ctx.enter_context(tc.tile_pool(name="psum", bufs=2, space="PSUM"))

        # ---- PE warmup ----
        # HAM keeps the PE at 1.2 GHz until ~3.4us of sustained activity;
        # burn the DMA/stats preamble warming it so the real matmuls run
        # at full clock. Any post-warmup PE idle >3.4us re-throttles.
        wz_l = consts.tile([128, 128], BF16, tag="wz_l")
        nc.vector.memset(wz_l, 0.0)
        wz_r = consts.tile([128, NCHUNK], BF16, tag="wz_r")
        nc.vector.memset(wz_r, 0.0)
        wps = psum.tile([128, NCHUNK], F32, tag="sp")
        for _ in range(NWARM):
            nc.tensor.matmul(out=wps, lhsT=wz_l, rhs=wz_r, start=True, stop=True)

        # ---- input loads: x (bf16, ones row baked by host) + const block ----
        xc = work.tile([C + 1, N], BF16, tag="xc")
        stats = small.tile([C, 6, 6], F32, tag="stats")
        for sub in range(6):
            eng = nc.sync if sub % 2 == 0 else nc.scalar
            eng.dma_start(
                out=xc[:, sub * 512 : (sub + 1) * 512],
                in_=xb_d[:, sub * 512 : (sub + 1) * 512],
            )
            nc.vector.bn_stats(
                out=stats[:, sub, :], in_=xc[0:C, sub * 512 : (sub + 1) * 512]
            )
        cblk = consts.tile([C + 1, CB_W], F32, tag="cblk")
        nc.sync.dma_start(out=cblk, in_=cblk_d[:, :])
        gnw = cblk[0:C, 320:321]
        gmat = cblk[0:C, 322:386]
        magic = consts.tile([C, 1], I32, tag="magic")
        nc.vector.memset(magic, 0x5F3759DF)
        shift1 = consts.tile([C, 1], I32, tag="shift1")
        nc.vector.memset(shift1, 1)

        # ---- GroupNorm statistics ----
        mv = small.tile([C, 2], F32, tag="mv")
        nc.vector.bn_aggr(out=mv, in_=stats)
        # stat2: col0 = mean_c, col1 = mean_c^2 + var_c (= E[x_c^2])
        stat2 = small.tile([C, 2], F32, tag="stat2")
        nc.vector.tensor_copy(out=stat2[:, 0:1], in_=mv[:, 0:1])
        nc.vector.tensor_mul(out=stat2[:, 1:2], in0=mv[:, 0:1], in1=mv[:, 0:1])
        nc.vector.tensor_add(out=stat2[:, 1:2], in0=stat2[:, 1:2], in1=mv[:, 1:2])
        # cross-partition group sums in one PE matmul against the 0/1
        # group-membership matrix (PE is only running warmup spam here)
        gsps = psum.tile([C, 2], F32, tag="pv", name="gsps")
        nc.tensor.matmul(out=gsps, lhsT=gmat, rhs=stat2, start=True, stop=True)
        gsum = small.tile([C, 2], F32, tag="gsum")
        nc.vector.tensor_copy(out=gsum, in_=gsps)
        gmean = small.tile([C, 1], F32, tag="gmean")
        nc.vector.tensor_scalar_mul(out=gmean, in0=gsum[:, 0:1], scalar1=1.0 / DH)
        # ve = var + eps = E[x^2] - mean^2 + eps
        msq = small.tile([C, 1], F32, tag="msq")
        nc.vector.tensor_mul(out=msq, in0=gmean, in1=gmean)
        ve = small.tile([C, 1], F32, tag="ve")
        nc.vector.tensor_scalar(
            out=ve, in0=gsum[:, 1:2], scalar1=1.0 / DH, scalar2=None, op0=ALU.mult
        )
        nc.vector.tensor_scalar(
            out=ve, in0=ve, scalar1=msq, scalar2=EPS,
            op0=ALU.subtract, op1=ALU.add,
        )
        # rstd = 1/sqrt(ve): fast-inverse-sqrt seed + 1 Newton iteration
        ish = small.tile([C, 1], I32, tag="ish")
        nc.vector.tensor_tensor(
            out=ish, in0=ve.bitcast(I32), in1=shift1, op=ALU.arith_shift_right
        )
        gint = small.tile([C, 1], I32, tag="gint")
        nc.vector.tensor_sub(out=gint, in0=magic, in1=ish)
        g = gint.bitcast(F32)
        t = small.tile([C, 1], F32, tag="t")
        for _ in range(2):
            nc.vector.tensor_mul(out=t, in0=g, in1=g)
            nc.vector.tensor_mul(out=t, in0=t, in1=ve)
            nc.vector.tensor_scalar(
                out=t, in0=t, scalar1=-0.5, scalar2=1.5, op0=ALU.mult, op1=ALU.add
            )
            nc.vector.tensor_mul(out=g, in0=g, in1=t)
        # sc65: rows 0:64 = rstd*gn_weight, row 64 = 1 so the host bias rows
        # of cblk pass through the augmented-scale multiply untouched
        sc65 = small.tile([C + 1, 1], F32, tag="sc65")
        nc.vector.memset(sc65[C : C + 1, :], 1.0)
        nc.vector.tensor_mul(out=sc65[0:C, :], in0=g, in1=gnw)

        # augmented weight tiles: row C = gn_bias contribution (host-built,
        # rides row 64 of cblk and passes through the x1 scale row)
        wq4a = work.tile([C + 1, 128], BF16, tag="wq4a")
        wk4a = work.tile([C + 1, 128], BF16, tag="wk4a")
        mvoa = work.tile([C + 1, C], BF16, tag="mvoa")
        nc.vector.tensor_scalar_mul(out=wq4a, in0=cblk[:, 0:128], scalar1=sc65)
        nc.vector.tensor_scalar_mul(out=wk4a, in0=cblk[:, 128:256], scalar1=sc65)
        nc.vector.tensor_scalar_mul(out=mvoa, in0=cblk[:, 256:320], scalar1=sc65)

        # mean-fold correction columns: mm2 = [[-m; 0], [m; 0]] fp32 (the
        # hq/hk/hg bias rows are already added by the ones-row in xc);
        # cps col0 = -W_q,s m (ACT bias form), col1 = +W_k,s m (DVE
        # subtract form), col2 = corr_g (rows 0:64, shipped to host)
        mm2 = small.tile([C + 1, 2], BF16, tag="mm2")
        nc.vector.tensor_scalar_mul(out=mm2[0:C, 0:1], in0=gmean, scalar1=-1.0)
        nc.vector.tensor_copy(out=mm2[0:C, 1:2], in_=gmean)
        nc.vector.memset(mm2[C : C + 1, :], 0.0)
        cps = psum.tile([128, 3], F32, tag="pv", name="cps")
        nc.tensor.matmul(out=cps[:, 0:1], lhsT=wq4a, rhs=mm2[:, 0:1], start=True, stop=True)
        nc.tensor.matmul(out=cps[:, 1:2], lhsT=wk4a, rhs=mm2[:, 1:2], start=True, stop=True)
        nc.tensor.matmul(out=cps[0:C, 2:3], lhsT=mvoa, rhs=mm2[:, 1:2], start=True, stop=True)
        corr = small.tile([128, 3], F32, tag="corr")
        nc.vector.tensor_copy(out=corr, in_=cps)

        # ---- Q/K (4x replicated along partition strips) ----
        q4 = work.tile([128, N], BF16, tag="q4")
        k4 = work.tile([128, N], BF16, tag="k4")

        def emit_proj_half(wmat, half):
            ps = psum.tile([128, PACK * NCHUNK], F32, tag="sp", name="ps")
            for cc in range(3):
                ic = half * 3 + cc
                nc.tensor.matmul(
                    out=ps[:, cc * NCHUNK : (cc + 1) * NCHUNK],
                    lhsT=wmat,
                    rhs=xc[:, ic * NCHUNK : (ic + 1) * NCHUNK],
                    start=True,
                    stop=True,
                )
            return ps

        def copy_k(dst_sl, src):
            # k4 = kps - corr_k, fused into the PSUM->SBUF cast
            nc.vector.tensor_scalar(
                out=dst_sl, in0=src, scalar1=corr[:, 1:2], scalar2=None,
                op0=ALU.subtract,
            )

        def copy_q(dst_sl, src):
            # q4 = qps + (-corr_q), fused into the PSUM->SBUF cast
            nc.scalar.activation(
                out=dst_sl, in_=src, func=ACTF.Identity, bias=corr[:, 0:1]
            )

        # All four projection halves run in the preamble; k copies stream on
        # VectorE while q copies stream on ScalarE so neither engine carries
        # copy work into the exp phase. QK pack 0 needs k4[0:768], q4[0:512].
        kps0 = emit_proj_half(wk4a, 0)
        copy_k(k4[:, 0:768], kps0[:, 0:768])
        qps0 = emit_proj_half(wq4a, 0)
        copy_q(q4[:, 0:NCHUNK], qps0[:, 0:NCHUNK])
        copy_k(k4[:, 768:1536], kps0[:, 768:1536])
        copy_q(q4[:, NCHUNK:1536], qps0[:, NCHUNK:1536])
        kps1 = emit_proj_half(wk4a, 1)
        copy_k(k4[:, 1536:N], kps1[:, :])
        qps1 = emit_proj_half(wq4a, 1)
        copy_q(q4[:, 1536:N], qps1[:, :])

        gsb = work.tile([128, NJB, C + 1], BF16, tag="gsb")

        def emit_qk(ic, jg, sp):
            for tt in range(PACK):
                jb = jg * PACK + tt
                nc.tensor.matmul(
                    out=sp[:, tt * NCHUNK : (tt + 1) * NCHUNK],
                    lhsT=k4[32 * tt : 32 * tt + DH, jb * JBLK : (jb + 1) * JBLK],
                    rhs=q4[32 * tt : 32 * tt + DH, ic * NCHUNK : (ic + 1) * NCHUNK],
                    start=True,
                    stop=True,
                    tile_position=(32 * tt, 0),
                )

        # G[j, 0:64] = (w_out_h @ w_v_h @ norm)^T blocks ; G[j, 64] = 1.
        # All 24 G matmuls run in the preamble (6 blocks per sp-slot pass)
        # so chunk 0 sees the same clean pack pipeline as every other chunk.
        nc.vector.memset(gsb[:, :, C : C + 1], 1.0)
        for gq in range(4):
            gps = psum.tile([128, 2 * PACK, C], F32, tag="sp", name="gps")
            for tt in range(2 * PACK):
                jb = gq * 2 * PACK + tt
                nc.tensor.matmul(
                    out=gps[:, tt, :],
                    lhsT=xc[:, jb * JBLK : (jb + 1) * JBLK],
                    rhs=mvoa,
                    start=True,
                    stop=True,
                )
            nc.vector.tensor_copy(
                out=gsb[:, gq * 2 * PACK : (gq + 1) * 2 * PACK, 0:C], in_=gps
            )

        # ship corr_g' to the host (fold applied as num/den - corr on host)
        cg_sb = small.tile([C, 1], F32, tag="cg_sb")
        nc.vector.tensor_copy(out=cg_sb, in_=corr[0:C, 2:3])
        nc.scalar.dma_start(out=out_d[0:C, N : N + 1], in_=cg_sb)

        # ---- main attention loop ----
        # Emission order per pack n: QK(n) first, then the exp of pack n on
        # its engine, then the PV of pack n-2 (pending). Keeping the PV two
        # packs behind means by the time the PE FIFO reaches a PV, its exp
        # finished ~2 pack-periods ago -- no head-of-line blocking of the
        # QKs queued behind it, so both exp engines stream back-to-back.
        chunk_pv = [None] * NCH
        pending = []

        def emit_pv(ic, jg, ep):
            pv = chunk_pv[ic]
            for tt in range(PACK):
                jb = jg * PACK + tt
                nc.tensor.matmul(
                    out=pv,
                    lhsT=gsb[:, jb, :],
                    rhs=ep[:, tt * NCHUNK : (tt + 1) * NCHUNK],
                    start=(jg == 0 and tt == 0),
                    stop=(jg == NPACKS - 1 and tt == PACK - 1),
                )
            if jg == NPACKS - 1:
                ostage = opool.tile([C + 1, NCHUNK], F32, tag="ostage")
                nc.vector.tensor_copy(out=ostage, in_=pv)
                nc.sync.dma_start(
                    out=out_d[:, ic * NCHUNK : (ic + 1) * NCHUNK], in_=ostage
                )

        for ic in range(NCH):
            chunk_pv[ic] = psum.tile([C + 1, NCHUNK], F32, tag="pv", name="pv")
            for jg in range(NPACKS):
                g = ic * NPACKS + jg
                sp = psum.tile([128, PACK * NCHUNK], F32, tag="sp", name="sp")
                emit_qk(ic, jg, sp)
                ep = epool.tile([128, PACK * NCHUNK], BF16, tag="ep")
                if g in DVE_PACKS:
                    nc.vector.tensor_scalar(
                        out=ep.bitcast(I16), in0=sp, scalar1=ASH, scalar2=BSH,
                        op0=ALU.mult, op1=ALU.add,
                    )
                else:
                    nc.scalar.activation(out=ep, in_=sp, func=ACTF.Exp)
                pending.append((ic, jg, ep))
                while len(pending) > 2:
                    emit_pv(*pending.pop(0))
        while pending:
            emit_pv(*pending.pop(0))

    nc.compile()
    return nc


_prog_cache = {}


def _get_program():
    if "nc" not in _prog_cache:
        _prog_cache["nc"] = build_program()
    return _prog_cache["nc"]


def _make_in_maps(x, gn_weight, gn_bias, w_qkv, w_out):
    xf = np.ascontiguousarray(x.reshape(B, C, N), np.float32)
    gnb = gn_bias.reshape(C).astype(np.float64)
    gmat = np.kron(np.eye(NG, dtype=np.float32), np.ones((DH, DH), np.float32))
    xbs = []
    for b in range(B):
        xb = np.ones((C + 1, N), ml_dtypes.bfloat16)
        xb[0:C] = xf[b].astype(ml_dtypes.bfloat16)
        xbs.append(xb)
    in_maps = []
    for core in range(B * NH):
        b, h = divmod(core, NH)
        wq = w_qkv[h * DH : (h + 1) * DH, :]  # [16, 64]
        wk = w_qkv[C + h * DH : C + (h + 1) * DH, :]
        wv = w_qkv[2 * C + h * DH : 2 * C + (h + 1) * DH, :]
        wo = w_out[:, h * DH : (h + 1) * DH]  # [64, 16]
        wq4 = np.zeros((C, 128), np.float32)
        wk4 = np.zeros((C, 128), np.float32)
        for t in range(4):
            wq4[:, 32 * t : 32 * t + DH] = wq.T
            wk4[:, 32 * t : 32 * t + DH] = wk.T
        mvoT = (wo.astype(np.float64) @ wv.astype(np.float64)).T.astype(np.float32)
        hq = (wq4.astype(np.float64).T @ gnb).astype(np.float32)  # [128]
        hk = (wk4.astype(np.float64).T @ gnb).astype(np.float32)
        hg = (mvoT.astype(np.float64).T @ gnb).astype(np.float32)  # [64]
        cblk = np.zeros((C + 1, CB_W), np.float32)
        cblk[0:C, 0:128] = wq4
        cblk[0:C, 128:256] = wk4
        cblk[0:C, 256:320] = mvoT
        cblk[0:C, 320] = gn_weight.reshape(C).astype(np.float32)
        cblk[0:C, 322:386] = gmat
        cblk[C, 0:128] = hq
        cblk[C, 128:256] = hk
        cblk[C, 256:320] = hg
        cblk[C, 320] = 1.0
        in_maps.append({"xb": xbs[b], "cblk": cblk})
    return in_maps


def _combine(results, in_maps, x, b_out):
    xf = x.reshape(B, C, N).astype(np.float32)
    out = np.zeros((B, C, N), np.float32)
    for core in range(B * NH):
        b = core // NH
        o = np.asarray(results[core]["out"], np.float32)  # [65, N+1]
        corr_g = o[0:C, N]  # = mvoTs^T m (hg already in G via ones row)
        out[b] += o[0:C, 0:N] / o[C : C + 1, 0:N] - corr_g[:, None]
    out += b_out.astype(np.float32)[None, :, None] + xf
    return out.reshape(B, C, D_, H_, W_).astype(np.float32)


def kernel(x, gn_weight, gn_bias, w_qkv, w_out, b_out, **_ignored):
    x = np.asarray(x, np.float32)
    w_qkv = np.asarray(w_qkv, np.float32)
    w_out = np.asarray(w_out, np.float32)
    b_out = np.asarray(b_out, np.float32)
    gn_weight = np.asarray(gn_weight, np.float32)
    gn_bias = np.asarray(gn_bias, np.float32)

    nc = _get_program()
    in_maps = _make_in_maps(x, gn_weight, gn_bias, w_qkv, w_out)
    res = run_bass_kernel_spmd(nc, in_maps, core_ids=list(range(B * NH)))
    return _combine(res.results, in_maps, x, b_out)


if __name__ == "__main__":
    import reference

    inputs = {k: np.asarray(v) for k, v in reference.setup_inputs().items()}
    actual = kernel(**inputs)
    print("kernel output shape:", actual.shape, actual.dtype)


# revision 18
# speedup vs baseline: 1.0220x; 1.0220x over previous
"""GroupNorm + single-head-per-core attention + output projection for
nn_Attention_55697135894780 on 8 TRN2 NeuronCores.

Sharding: one (batch, head) pair per core (B=2 x NH=4 = 8 cores), no
cross-device communication. Each core computes, for its (b, h):

  xc     = raw x[b] in bf16 with a ones row (row 64), straight from DMA.
           GroupNorm's mean subtraction never touches the big tensor:
           it is folded into per-output-channel correction columns
           (corr = W_scaled @ mean - bias_row, one tiny matmul per
           projection) applied during the PSUM->SBUF copies (ScalarE
           Identity-bias / VectorE tensor_scalar-subtract), and for the
           G path deferred to the host via num/den linearity.
  s_c    = gn_weight_c / sqrt(group_var_c + eps); group sums via one PE
           matmul against a group-membership matrix (no shuffles).
  q4/k4  = replicated head projections                    [128, 3072] bf16
  S^T    = K^T Q computed j-on-partitions (no transposes anywhere)
  E      = exp(S^T) in bf16, split between ScalarE (table exp) and
           VectorE (one-instruction Schraudolph bit-trick) streaming
           concurrently from different PSUM banks
  out    = [w_out_h @ w_v_h @ norm ; 1]^T-weighted sum of E
           rows 0:64 = numerator, row 64 = softmax denominator;
           col N carries the host-side G correction vector.

Host combines: x + b_out + sum_h(num/den - corr_g), reshaped.
"""

import sys
from contextlib import ExitStack

import numpy as np
import ml_dtypes

sys.path.insert(0, "/opt/trn_rl_repo")

import concourse.bacc as bacc  # noqa: E402
import concourse.bass as bass  # noqa: E402
import concourse.tile as tile  # noqa: E402
from concourse import mybir  # noqa: E402
from concourse.bass_utils import run_bass_kernel_spmd  # noqa: E402

B, C, D_, H_, W_ = 2, 64, 12, 16, 16
N = D_ * H_ * W_  # 3072
NH, DH, NG = 4, 16, 4  # heads, head_dim, groups
EPS = 1e-5
F32 = mybir.dt.float32
BF16 = mybir.dt.bfloat16
I32 = mybir.dt.int32
I16 = mybir.dt.int16
ALU = mybir.AluOpType
ACTF = mybir.ActivationFunctionType

NCHUNK = 512
NCH = N // NCHUNK  # 6 i-chunks
JBLK = 128
NJB = N // JBLK  # 24 j-blocks
PACK = 3  # j-blocks per PSUM pack (3 banks; x2 buffers + 2 PV banks = 8)
NPACKS = NJB // PACK  # 8 packs per chunk, 48 total
NWARM = 10

# Global pack indices whose exp runs on VectorE (Schraudolph). Pattern
# A D A D A per 5 keeps adjacent packs mostly on different engines so the
# 2-slot PSUM rotation lets both engines stream concurrently.
DVE_PACKS = frozenset(g for g in range(NCH * NPACKS) if g % 5 in (1, 3))

# Schraudolph constants: bits of bf16(exp(S)) ~= int16(S*128/ln2 + 127*128 - CSH)
ASH = 128.0 / float(np.log(2.0))
CSH = 5.5
BSH = 127.0 * 128.0 - CSH

# const-block column layout (fp32 [65, 386]):
#   0:128 wq4 (row64 = hq), 128:256 wk4 (row64 = hk),
#   256:320 mvoT (row64 = hg), 320 gnw (row64 = 1), 321 pad,
#   322:386 gmat (group-membership 0/1, row64 = 0)
CB_W = 386
OUT_W = N + 1  # col N = corr_g vector (rows 0:64)


def build_program():
    nc = bacc.Bacc("TRN2", target_bir_lowering=False)

    xb_d = nc.dram_tensor("xb", [C + 1, N], BF16, kind="ExternalInput")
    cblk_d = nc.dram_tensor("cblk", [C + 1, CB_W], F32, kind="ExternalInput")
    out_d = nc.dram_tensor("out", [C + 1, OUT_W], F32, kind="ExternalOutput")

    with tile.TileContext(nc) as tc, ExitStack() as ctx:
        consts = ctx.enter_context(tc.tile_pool(name="consts", bufs=1))
        work = ctx.enter_context(tc.tile_pool(name="work", bufs=1))
        small = ctx.enter_context(tc.tile_pool(name="small", bufs=2))
        epool = ctx.enter_context(tc.tile_pool(name="epool", bufs=5))
        opool = ctx.enter_context(tc.tile_pool(name="opool", bufs=2))
        psum = ctx.enter_context(tc.tile_pool(name="psum", bufs=2, space="PSUM"))

        # ---- PE warmup ----
        # HAM keeps the PE at 1.2 GHz until ~3.4us of sustained activity;
        # burn the DMA/stats preamble warming it so the real matmuls run
        # at full clock. Any post-warmup PE idle >3.4us re-throttles.
        # warmup operand init on GpSimd (idle at start) so VectorE is free
        # for bn_stats the moment x chunks land
        wz_l = consts.tile([128, 128], BF16, tag="wz_l")
        nc.gpsimd.memset(wz_l, 0.0)
        wz_r = consts.tile([128, NCHUNK], BF16, tag="wz_r")
        nc.gpsimd.memset(wz_r, 0.0)
        wps = psum.tile([128, NCHUNK], F32, tag="sp")
        for _ in range(NWARM):
            nc.tensor.matmul(out=wps, lhsT=wz_l, rhs=wz_r, start=True, stop=True)

        # ---- input loads: x (bf16, ones row baked by host) + const block ----
        xc = work.tile([C + 1, N], BF16, tag="xc")
        stats = small.tile([C, 6, 6], F32, tag="stats")
        for sub in range(6):
            eng = nc.sync if sub % 2 == 0 else nc.scalar
            eng.dma_start(
                out=xc[:, sub * 512 : (sub + 1) * 512],
                in_=xb_d[:, sub * 512 : (sub + 1) * 512],
            )
            nc.vector.bn_stats(
                out=stats[:, sub, :], in_=xc[0:C, sub * 512 : (sub + 1) * 512]
            )
        cblk = consts.tile([C + 1, CB_W], F32, tag="cblk")
        nc.sync.dma_start(out=cblk, in_=cblk_d[:, :])
        gnw = cblk[0:C, 320:321]
        gmat = cblk[0:C, 322:386]
        magic = consts.tile([C, 1], I32, tag="magic")
        nc.vector.memset(magic, 0x5F3759DF)
        shift1 = consts.tile([C, 1], I32, tag="shift1")
        nc.vector.memset(shift1, 1)

        # ---- GroupNorm statistics ----
        mv = small.tile([C, 2], F32, tag="mv")
        nc.vector.bn_aggr(out=mv, in_=stats)
        # stat2: col0 = mean_c, col1 = mean_c^2 + var_c (= E[x_c^2])
        stat2 = small.tile([C, 2], F32, tag="stat2")
        nc.vector.tensor_copy(out=stat2[:, 0:1], in_=mv[:, 0:1])
        nc.vector.tensor_mul(out=stat2[:, 1:2], in0=mv[:, 0:1], in1=mv[:, 0:1])
        nc.vector.tensor_add(out=stat2[:, 1:2], in0=stat2[:, 1:2], in1=mv[:, 1:2])
        # cross-partition group sums in one PE matmul against the 0/1
        # group-membership matrix (PE is only running warmup spam here)
        gsps = psum.tile([C, 2], F32, tag="pv", name="gsps")
        nc.tensor.matmul(out=gsps, lhsT=gmat, rhs=stat2, start=True, stop=True)
        gsum = small.tile([C, 2], F32, tag="gsum")
        nc.vector.tensor_copy(out=gsum, in_=gsps)
        gmean = small.tile([C, 1], F32, tag="gmean")
        nc.vector.tensor_scalar_mul(out=gmean, in0=gsum[:, 0:1], scalar1=1.0 / DH)
        # ve = var + eps = E[x^2] - mean^2 + eps
        msq = small.tile([C, 1], F32, tag="msq")
        nc.vector.tensor_mul(out=msq, in0=gmean, in1=gmean)
        ve = small.tile([C, 1], F32, tag="ve")
        nc.vector.tensor_scalar(
            out=ve, in0=gsum[:, 1:2], scalar1=1.0 / DH, scalar2=None, op0=ALU.mult
        )
        nc.vector.tensor_scalar(
            out=ve, in0=ve, scalar1=msq, scalar2=EPS,
            op0=ALU.subtract, op1=ALU.add,
        )
        # rstd = 1/sqrt(ve): fast-inverse-sqrt seed + 1 Newton iteration
        ish = small.tile([C, 1], I32, tag="ish")
        nc.vector.tensor_tensor(
            out=ish, in0=ve.bitcast(I32), in1=shift1, op=ALU.arith_shift_right
        )
        gint = small.tile([C, 1], I32, tag="gint")
        nc.vector.tensor_sub(out=gint, in0=magic, in1=ish)
        g = gint.bitcast(F32)
        t = small.tile([C, 1], F32, tag="t")
        for _ in range(2):
            nc.vector.tensor_mul(out=t, in0=g, in1=g)
            nc.vector.tensor_mul(out=t, in0=t, in1=ve)
            nc.vector.tensor_scalar(
                out=t, in0=t, scalar1=-0.5, scalar2=1.5, op0=ALU.mult, op1=ALU.add
            )
            nc.vector.tensor_mul(out=g, in0=g, in1=t)
        # sc65: rows 0:64 = rstd*gn_weight, row 64 = 1 so the host bias rows
        # of cblk pass through the augmented-scale multiply untouched
        sc65 = small.tile([C + 1, 1], F32, tag="sc65")
        nc.vector.memset(sc65[C : C + 1, :], 1.0)
        nc.vector.tensor_mul(out=sc65[0:C, :], in0=g, in1=gnw)

        # augmented weight tiles: row C = gn_bias contribution (host-built,
        # rides row 64 of cblk and passes through the x1 scale row)
        wq4a = work.tile([C + 1, 128], BF16, tag="wq4a")
        wk4a = work.tile([C + 1, 128], BF16, tag="wk4a")
        mvoa = work.tile([C + 1, C], BF16, tag="mvoa")
        nc.vector.tensor_scalar_mul(out=wq4a, in0=cblk[:, 0:128], scalar1=sc65)
        nc.vector.tensor_scalar_mul(out=wk4a, in0=cblk[:, 128:256], scalar1=sc65)
        nc.vector.tensor_scalar_mul(out=mvoa, in0=cblk[:, 256:320], scalar1=sc65)

        # mean-fold correction columns: mm2 = [[-m; 0], [m; 0]] fp32 (the
        # hq/hk/hg bias rows are already added by the ones-row in xc);
        # cps col0 = -W_q,s m (ACT bias form), col1 = +W_k,s m (DVE
        # subtract form), col2 = corr_g (rows 0:64, shipped to host)
        mm2 = small.tile([C + 1, 2], BF16, tag="mm2")
        nc.vector.tensor_scalar_mul(out=mm2[0:C, 0:1], in0=gmean, scalar1=-1.0)
        nc.vector.tensor_copy(out=mm2[0:C, 1:2], in_=gmean)
        nc.vector.memset(mm2[C : C + 1, :], 0.0)
        cps = psum.tile([128, 3], F32, tag="pv", name="cps")
        nc.tensor.matmul(out=cps[:, 0:1], lhsT=wq4a, rhs=mm2[:, 0:1], start=True, stop=True)
        nc.tensor.matmul(out=cps[:, 1:2], lhsT=wk4a, rhs=mm2[:, 1:2], start=True, stop=True)
        nc.tensor.matmul(out=cps[0:C, 2:3], lhsT=mvoa, rhs=mm2[:, 1:2], start=True, stop=True)
        corr = small.tile([128, 3], F32, tag="corr")
        nc.vector.tensor_copy(out=corr, in_=cps)

        # ---- Q/K (4x replicated along partition strips) ----
        q4 = work.tile([128, N], BF16, tag="q4")
        k4 = work.tile([128, N], BF16, tag="k4")

        def emit_proj_half(wmat, half):
            ps = psum.tile([128, PACK * NCHUNK], F32, tag="sp", name="ps")
            for cc in range(3):
                ic = half * 3 + cc
                nc.tensor.matmul(
                    out=ps[:, cc * NCHUNK : (cc + 1) * NCHUNK],
                    lhsT=wmat,
                    rhs=xc[:, ic * NCHUNK : (ic + 1) * NCHUNK],
                    start=True,
                    stop=True,
                )
            return ps

        def copy_k(dst_sl, src):
            # k4 = kps - corr_k, fused into the PSUM->SBUF cast
            nc.vector.tensor_scalar(
                out=dst_sl, in0=src, scalar1=corr[:, 1:2], scalar2=None,
                op0=ALU.subtract,
            )

        def copy_q(dst_sl, src):
            # q4 = qps + (-corr_q), fused into the PSUM->SBUF cast
            nc.scalar.activation(
                out=dst_sl, in_=src, func=ACTF.Identity, bias=corr[:, 0:1]
            )

        # All four projection halves run in the preamble; k copies stream on
        # VectorE while q copies stream on ScalarE so neither engine carries
        # copy work into the exp phase. QK pack 0 needs k4[0:768], q4[0:512].
        kps0 = emit_proj_half(wk4a, 0)
        copy_k(k4[:, 0:768], kps0[:, 0:768])
        qps0 = emit_proj_half(wq4a, 0)
        copy_q(q4[:, 0:NCHUNK], qps0[:, 0:NCHUNK])
        copy_k(k4[:, 768:1536], kps0[:, 768:1536])
        copy_q(q4[:, NCHUNK:1536], qps0[:, NCHUNK:1536])
        kps1 = emit_proj_half(wk4a, 1)
        copy_k(k4[:, 1536:N], kps1[:, :])
        qps1 = emit_proj_half(wq4a, 1)
        copy_q(q4[:, 1536:N], qps1[:, :])

        gsb = work.tile([128, NJB, C + 1], BF16, tag="gsb")

        def emit_qk(ic, jg, sp):
            for tt in range(PACK):
                jb = jg * PACK + tt
                nc.tensor.matmul(
                    out=sp[:, tt * NCHUNK : (tt + 1) * NCHUNK],
                    lhsT=k4[32 * tt : 32 * tt + DH, jb * JBLK : (jb + 1) * JBLK],
                    rhs=q4[32 * tt : 32 * tt + DH, ic * NCHUNK : (ic + 1) * NCHUNK],
                    start=True,
                    stop=True,
                    tile_position=(32 * tt, 0),
                )

        # G[j, 0:64] = (w_out_h @ w_v_h @ norm)^T blocks ; G[j, 64] = 1.
        # All 24 G matmuls run in the preamble (6 blocks per sp-slot pass)
        # so chunk 0 sees the same clean pack pipeline as every other chunk.
        nc.vector.memset(gsb[:, :, C : C + 1], 1.0)
        for gq in range(4):
            gps = psum.tile([128, 2 * PACK, C], F32, tag="sp", name="gps")
            for tt in range(2 * PACK):
                jb = gq * 2 * PACK + tt
                nc.tensor.matmul(
                    out=gps[:, tt, :],
                    lhsT=xc[:, jb * JBLK : (jb + 1) * JBLK],
                    rhs=mvoa,
                    start=True,
                    stop=True,
                )
            nc.vector.tensor_copy(
                out=gsb[:, gq * 2 * PACK : (gq + 1) * 2 * PACK, 0:C], in_=gps
            )

        # ship corr_g' to the host (fold applied as num/den - corr on host)
        cg_sb = small.tile([C, 1], F32, tag="cg_sb")
        nc.vector.tensor_copy(out=cg_sb, in_=corr[0:C, 2:3])
        nc.scalar.dma_start(out=out_d[0:C, N : N + 1], in_=cg_sb)

        # ---- main attention loop ----
        # Emission order per pack n: QK(n) first, then the exp of pack n on
        # its engine, then the PV of pack n-2 (pending). Keeping the PV two
        # packs behind means by the time the PE FIFO reaches a PV, its exp
        # finished ~2 pack-periods ago -- no head-of-line blocking of the
        # QKs queued behind it, so both exp engines stream back-to-back.
        chunk_pv = [None] * NCH
        pending = []

        def emit_pv(ic, jg, ep):
            pv = chunk_pv[ic]
            for tt in range(PACK):
                jb = jg * PACK + tt
                nc.tensor.matmul(
                    out=pv,
                    lhsT=gsb[:, jb, :],
                    rhs=ep[:, tt * NCHUNK : (tt + 1) * NCHUNK],
                    start=(jg == 0 and tt == 0),
                    stop=(jg == NPACKS - 1 and tt == PACK - 1),
                )
            if jg == NPACKS - 1:
                ostage = opool.tile([C + 1, NCHUNK], F32, tag="ostage")
                nc.vector.tensor_copy(out=ostage, in_=pv)
                nc.sync.dma_start(
                    out=out_d[:, ic * NCHUNK : (ic + 1) * NCHUNK], in_=ostage
                )

        for ic in range(NCH):
            chunk_pv[ic] = psum.tile([C + 1, NCHUNK], F32, tag="pv", name="pv")
            for jg in range(NPACKS):
                g = ic * NPACKS + jg
                sp = psum.tile([128, PACK * NCHUNK], F32, tag="sp", name="sp")
                emit_qk(ic, jg, sp)
                ep = epool.tile([128, PACK * NCHUNK], BF16, tag="ep")
                if g in DVE_PACKS:
                    nc.vector.tensor_scalar(
                        out=ep.bitcast(I16), in0=sp, scalar1=ASH, scalar2=BSH,
                        op0=ALU.mult, op1=ALU.add,
                    )
                else:
                    nc.scalar.activation(out=ep, in_=sp, func=ACTF.Exp)
                pending.append((ic, jg, ep))
                while len(pending) > 2:
                    emit_pv(*pending.pop(0))
        while pending:
            emit_pv(*pending.pop(0))

    nc.compile()
    return nc


_prog_cache = {}


def _get_program():
    if "nc" not in _prog_cache:
        _prog_cache["nc"] = build_program()
    return _prog_cache["nc"]


def _make_in_maps(x, gn_weight, gn_bias, w_qkv, w_out):
    xf = np.ascontiguousarray(x.reshape(B, C, N), np.float32)
    gnb = gn_bias.reshape(C).astype(np.float64)
    gmat = np.kron(np.eye(NG, dtype=np.float32), np.ones((DH, DH), np.float32))
    xbs = []
    for b in range(B):
        xb = np.ones((C + 1, N), ml_dtypes.bfloat16)
        xb[0:C] = xf[b].astype(ml_dtypes.bfloat16)
        xbs.append(xb)
    in_maps = []
    for core in range(B * NH):
        b, h = divmod(core, NH)
        wq = w_qkv[h * DH : (h + 1) * DH, :]  # [16, 64]
        wk = w_qkv[C + h * DH : C + (h + 1) * DH, :]
        wv = w_qkv[2 * C + h * DH : 2 * C + (h + 1) * DH, :]
        wo = w_out[:, h * DH : (h + 1) * DH]  # [64, 16]
        wq4 = np.zeros((C, 128), np.float32)
        wk4 = np.zeros((C, 128), np.float32)
        for t in range(4):
            wq4[:, 32 * t : 32 * t + DH] = wq.T
            wk4[:, 32 * t : 32 * t + DH] = wk.T
        mvoT = (wo.astype(np.float64) @ wv.astype(np.float64)).T.astype(np.float32)
        hq = (wq4.astype(np.float64).T @ gnb).astype(np.float32)  # [128]
        hk = (wk4.astype(np.float64).T @ gnb).astype(np.float32)
        hg = (mvoT.astype(np.float64).T @ gnb).astype(np.float32)  # [64]
        cblk = np.zeros((C + 1, CB_W), np.float32)
        cblk[0:C, 0:128] = wq4
        cblk[0:C, 128:256] = wk4
        cblk[0:C, 256:320] = mvoT
        cblk[0:C, 320] = gn_weight.reshape(C).astype(np.float32)
        cblk[0:C, 322:386] = gmat
        cblk[C, 0:128] = hq
        cblk[C, 128:256] = hk
        cblk[C, 256:320] = hg
        cblk[C, 320] = 1.0
        in_maps.append({"xb": xbs[b], "cblk": cblk})
    return in_maps


def _combine(results, in_maps, x, b_out):
    xf = x.reshape(B, C, N).astype(np.float32)
    out = np.zeros((B, C, N), np.float32)
    for core in range(B * NH):
        b = core // NH
        o = np.asarray(results[core]["out"], np.float32)  # [65, N+1]
        corr_g = o[0:C, N]  # = mvoTs^T m (hg already in G via ones row)
        out[b] += o[0:C, 0:N] / o[C : C + 1, 0:N] - corr_g[:, None]
    out += b_out.astype(np.float32)[None, :, None] + xf
    return out.reshape(B, C, D_, H_, W_).astype(np.float32)


def kernel(x, gn_weight, gn_bias, w_qkv, w_out, b_out, **_ignored):
    x = np.asarray(x, np.float32)
    w_qkv = np.asarray(w_qkv, np.float32)
    w_out = np.asarray(w_out, np.float32)
    b_out = np.asarray(b_out, np.float32)
    gn_weight = np.asarray(gn_weight, np.float32)
    gn_bias = np.asarray(gn_bias, np.float32)

    nc = _get_program()
    in_maps = _make_in_maps(x, gn_weight, gn_bias, w_qkv, w_out)
    res = run_bass_kernel_spmd(nc, in_maps, core_ids=list(range(B * NH)))
    return _combine(res.results, in_maps, x, b_out)


if __name__ == "__main__":
    import reference

    inputs = {k: np.asarray(v) for k, v in reference.setup_inputs().items()}
    actual = kernel(**inputs)
    print("kernel output shape:", actual.shape, actual.dtype)
